# revision 17
# baseline (speedup 1.0000x reference)
"""Multihead self-attention (T=1024, B=4, E=1024, H=16) on 8 TRN2 NeuronCores.

Sharding: head-parallel. Core i owns heads {2i, 2i+1} == E-rows [128i, 128i+128)
of Wq/Wk/Wv, and all 4 batches. No cross-core communication.

Per-core dataflow (all "transposed" layouts, d on partitions):
  qT/kT/vT [128, B*T] = W_slice @ query.T   (PE, float32r, K=E in 8 chunks)
  per (b, head, t-chunk of 512):
    scoresT [s=128, t<=512] = kT_tile.T-free matmul; causal tiles above the
    diagonal are skipped entirely, diagonal tiles are column-sliced.
    probs = Exp(scoresT + causal_mask + padding_bias)  (ACT, padding as
    per-partition bias; both heads share one [128, 1024] ACT op)
    outT[65, 512] += va_tile[128, 65].T @ probs  where va has a ones column,
    so row 64 accumulates the softmax denominator.
  normalize: DMA-broadcast denominator row, DVE reciprocal + multiply.
Host gathers [128, B, T] per-core outputs -> [T, B, E].
"""

import numpy as np

T, B, E, H = 1024, 4, 1024, 16
D = 64  # head dim
NCORES = 8
HPC = H // NCORES  # heads per core = 2
DS = HPC * D  # per-core E-slice = 128
TB = T * B
NEG = -1.0e30
SCALE = D**-0.5

_COMPILED = {}


def _build_program():
    import concourse.bacc as bacc
    import concourse.mybir as mybir
    import concourse.tile as tile
    from concourse.masks import make_identity

    f32 = mybir.dt.float32
    f32r = mybir.dt.float32r
    AF = mybir.ActivationFunctionType
    ALU = mybir.AluOpType

    nc = bacc.Bacc("TRN2", target_bir_lowering=False, debug=False,
                   num_devices=NCORES)

    qt = nc.dram_tensor("qt", [E, B, T], f32r, kind="ExternalInput").ap()
    wq = nc.dram_tensor("wq", [E, DS], f32r, kind="ExternalInput").ap()
    wk = nc.dram_tensor("wk", [E, DS], f32r, kind="ExternalInput").ap()
    wv = nc.dram_tensor("wv", [E, DS], f32r, kind="ExternalInput").ap()
    bq = nc.dram_tensor("bq", [DS, 1], f32, kind="ExternalInput").ap()
    bk = nc.dram_tensor("bk", [DS, 1], f32, kind="ExternalInput").ap()
    bv = nc.dram_tensor("bv", [DS, 1], f32, kind="ExternalInput").ap()
    cm = nc.dram_tensor("cm", [4, 128, 512], f32, kind="ExternalInput").ap()
    pad = nc.dram_tensor("pad", [128, B * 8], f32, kind="ExternalInput").ap()
    onesd = nc.dram_tensor("ones", [128, 16], f32r, kind="ExternalInput").ap()
    out = nc.dram_tensor("out", [DS, B, T], f32, kind="ExternalOutput").ap()

    NJ = TB // 512  # 8 token chunks of 512; chunk j covers (b=j//2, half=j%2)

    with tile.TileContext(nc) as tc:
        with (
            tc.tile_pool(name="consts", bufs=1) as consts,
            tc.tile_pool(name="qkv", bufs=NJ) as qkv,
            tc.tile_pool(name="va", bufs=B) as vap,
            tc.tile_pool(name="probs", bufs=3) as probsp,
            tc.tile_pool(name="outsb", bufs=3) as outsb,
            tc.tile_pool(name="bcast", bufs=3) as bcastp,
        ):
            ident = consts.tile([128, 128], f32, name="ident")
            make_identity(nc, ident[:])
            w_sb = {}
            b_sb = {}
            for nm, wdr, bdr in (("q", wq, bq), ("k", wk, bk), ("v", wv, bv)):
                wt = consts.tile([128, 8, DS], f32r, name=f"w{nm}s")
                nc.sync.dma_start(wt[:], wdr.rearrange("(c p) m -> p c m", p=128))
                w_sb[nm] = wt
                bt = consts.tile([DS, 1], f32, name=f"b{nm}s")
                nc.sync.dma_start(bt[:], bdr)
                b_sb[nm] = bt
            cm_sb = consts.tile([128, 4, 512], f32, name="cms")
            nc.sync.dma_start(cm_sb[:], cm.rearrange("d p f -> p d f"))
            pad_sb = consts.tile([128, B * 8], f32, name="pads")
            nc.sync.dma_start(pad_sb[:], pad)

            # ---- Phase 1: QKV projection into transposed layout ----
            qkv_t = {"q": [], "k": [], "v": []}
            ph1 = tc.tile_pool(name="rhs", bufs=10)
            rhsp = ph1.__enter__()
            ph1b = tc.tile_pool(name="psA", bufs=2, space="PSUM")
            psA = ph1b.__enter__()
            for j in range(NJ):
                qu = []
                for e in range(8):
                    qtile = rhsp.tile([128, 512], f32r, tag="qu", name=f"qu{j}_{e}")
                    nc.sync.dma_start(
                        qtile[:],
                        qt.rearrange("e b t -> e (b t)")[
                            e * 128:(e + 1) * 128, j * 512:(j + 1) * 512
                        ],
                    )
                    qu.append(qtile)
                for nm in ("q", "k", "v"):
                    ps = psA.tile([128, 512], f32, tag="proj", name=f"ps{nm}{j}")
                    for e in range(8):
                        nc.tensor.matmul(
                            ps[:],
                            w_sb[nm][:, e, :],
                            qu[e][:],
                            start=(e == 0),
                            stop=(e == 7),
                        )
                    dst = qkv.tile([128, 512], f32r if nm != "v" else f32, tag=f"{nm}t", name=f"{nm}t{j}")
                    if nm == "q":
                        nc.vector.tensor_scalar(
                            dst[:], ps[:], b_sb[nm][:], SCALE,
                            op0=ALU.add, op1=ALU.mult,
                        )
                    else:
                        nc.vector.tensor_scalar(
                            dst[:], ps[:], b_sb[nm][:], None, op0=ALU.add,
                        )
                    qkv_t[nm].append(dst)

            ph1b.__exit__(None, None, None)
            ph1.__exit__(None, None, None)

            # ---- Phase 1.5: transpose vT -> va ([s, d] with ones column) ----
            va_t = []
            for b in range(B):
                va = vap.tile([128, 8, 2, 65], f32r, tag="va", name=f"va{b}")
                nc.sync.dma_start(
                    va[:, :, :, 64:65],
                    onesd.rearrange("p (a b c) -> p a b c", a=8, b=2, c=1),
                )
                va_t.append(va)
            ph15 = tc.tile_pool(name="psTr", bufs=2, space="PSUM")
            psTr = ph15.__enter__()
            for b in range(B):
                for p in range(8):
                    j = 2 * b + p // 4
                    off = (p % 4) * 128
                    tp = psTr.tile([128, 128], f32, tag="tr", name=f"tr{b}_{p}")
                    nc.tensor.transpose(
                        tp[:], qkv_t["v"][j][:, off:off + 128], ident[:]
                    )
                    nc.vector.tensor_copy(
                        va_t[b][:, p, :, 0:64],
                        tp[:].rearrange("p (two sub) -> p two sub", two=2),
                    )
            ph15.__exit__(None, None, None)

            # ---- Phase 2: attention ----
            ph2 = tc.tile_pool(name="psS", bufs=2, space="PSUM")
            psS = ph2.__enter__()
            ph2b = tc.tile_pool(name="psO", bufs=4, space="PSUM")
            psO = ph2b.__enter__()
            ph2c = tc.tile_pool(name="drp", bufs=3, space="DRAM")
            drp = ph2c.__enter__()
            for b in range(B):
                for c in range(2):  # t-chunks of 512
                    j = 2 * b + c
                    po = []
                    for hl in range(HPC):
                        pot = psO.tile([65, 512], f32, tag="po",
                                       name=f"po{b}_{c}_{hl}")
                        po.append(pot)
                    ntile = 4 * (c + 1)
                    for p in range(ntile):
                        dlt = p - 4 * c
                        w0 = 128 * dlt if dlt > 0 else 0
                        jk = 2 * b + p // 4
                        offk = (p % 4) * 128
                        ss = psS.tile([128, 2, 512], f32, tag="sc",
                                      name=f"sc{b}_{c}_{p}")
                        for hl in range(HPC):
                            nc.tensor.matmul(
                                ss[:, hl, w0:512],
                                qkv_t["k"][jk][hl * 64:(hl + 1) * 64,
                                               offk:offk + 128],
                                qkv_t["q"][j][hl * 64:(hl + 1) * 64,
                                              w0:512],
                                start=True,
                                stop=True,
                            )
                        if dlt >= 0:
                            nc.vector.tensor_tensor(
                                ss[:, :, w0:512],
                                ss[:, :, w0:512],
                                cm_sb[:, dlt, None, w0:512].to_broadcast(
                                    (128, 2, 512 - w0)
                                ),
                                ALU.add,
                            )
                        pr = probsp.tile([128, 2, 512], f32r, tag="pr",
                                         name=f"pr{b}_{c}_{p}")
                        nc.scalar.activation(
                            pr[:, :, w0:512],
                            ss[:, :, w0:512],
                            AF.Exp,
                            bias=pad_sb[:, b * 8 + p:b * 8 + p + 1],
                            scale=1.0,
                        )
                        for hl in range(HPC):
                            nc.tensor.matmul(
                                po[hl][:, w0:512],
                                va_t[b][:, p, hl, :],
                                pr[:, hl, w0:512],
                                start=(p == 0),
                                stop=(p == ntile - 1),
                            )
                    for hl in range(HPC):
                        # denominator row PSUM->SBUF, then DMA-broadcast to
                        # 64 partitions
                        rc = bcastp.tile([1, 512], f32, tag="rc",
                                         name=f"rc{b}_{c}_{hl}")
                        nc.scalar.activation(rc[:], po[hl][64:65, :], AF.Copy)
                        dn = drp.tile([1, 512], f32, tag="dn",
                                      name=f"dn{b}_{c}_{hl}")
                        nc.sync.dma_start(dn[:], rc[:])
                        bcs = bcastp.tile([64, 512], f32, tag="bc",
                                          name=f"bc{b}_{c}_{hl}")
                        nc.sync.dma_start(bcs[:], dn[:].to_broadcast((64, 512)))
                        nc.vector.reciprocal(bcs[:], bcs[:])
                        of = outsb.tile([64, 512], f32, tag="of",
                                        name=f"of{b}_{c}_{hl}")
                        nc.vector.tensor_mul(of[:], po[hl][0:64, :], bcs[:])
                        nc.sync.dma_start(
                            out[hl * 64:(hl + 1) * 64, b,
                                c * 512:(c + 1) * 512],
                            of[:],
                        )
            ph2c.__exit__(None, None, None)
            ph2b.__exit__(None, None, None)
            ph2.__exit__(None, None, None)

    nc.compile()
    return nc


def _get_program():
    if "nc" not in _COMPILED:
        _COMPILED["nc"] = _build_program()
    return _COMPILED["nc"]


def _prepare_in_maps(query, key_padding_mask, attn_mask, Wq, bq, Wk, bk, Wv,
                     bv):
    query = np.asarray(query, dtype=np.float32)
    attn_mask = np.asarray(attn_mask, dtype=np.float32)
    kpm = np.asarray(key_padding_mask)
    Wq, Wk, Wv = (np.asarray(w, dtype=np.float32) for w in (Wq, Wk, Wv))
    bq, bk, bv = (np.asarray(x, dtype=np.float32) for x in (bq, bk, bv))

    qt = np.ascontiguousarray(query.transpose(2, 1, 0))  # [E, B, T]
    # causal masks for the 4 diagonal-crossing tile offsets, from attn_mask:
    # cmh[d][p, f] = attn_mask[f, 128 d + p], f in [0, 512)
    cmh = np.stack(
        [
            np.ascontiguousarray(attn_mask[:512, 128 * d:128 * (d + 1)].T)
            for d in range(4)
        ]
    )
    cmh = np.maximum(cmh, NEG)  # -inf -> -1e30 (exp underflows to exactly 0)
    padf = np.where(kpm, NEG, 0.0).astype(np.float32)  # [B, T]
    padh = np.ascontiguousarray(
        padf.reshape(B, 8, 128).transpose(2, 0, 1).reshape(128, B * 8)
    )

    in_maps = []
    for i in range(NCORES):
        rows = slice(i * DS, (i + 1) * DS)
        in_maps.append(
            {
                "qt": qt,
                "wq": np.ascontiguousarray(Wq[rows].T),
                "wk": np.ascontiguousarray(Wk[rows].T),
                "wv": np.ascontiguousarray(Wv[rows].T),
                "bq": np.ascontiguousarray(bq[rows, None]),
                "bk": np.ascontiguousarray(bk[rows, None]),
                "bv": np.ascontiguousarray(bv[rows, None]),
                "cm": cmh,
                "pad": padh,
                "ones": np.ones((128, 16), dtype=np.float32),
            }
        )
    return in_maps


def kernel(query, key, key_padding_mask, attn_mask, Wq, bq, Wk, bk, Wv, bv,
           num_heads):
    from concourse.bass_utils import run_bass_kernel_spmd

    assert int(num_heads) == H
    nc = _get_program()
    in_maps = _prepare_in_maps(query, key_padding_mask, attn_mask, Wq, bq, Wk,
                               bk, Wv, bv)
    res = run_bass_kernel_spmd(nc, in_maps, core_ids=list(range(NCORES)))
    full = np.concatenate(
        [res.results[i]["out"].transpose(2, 1, 0) for i in range(NCORES)],
        axis=2,
    )
    return full


# revision 20
# speedup vs baseline: 1.2207x; 1.2207x over previous
"""Multihead self-attention (T=1024, B=4, E=1024, H=16) on 8 TRN2 NeuronCores.

Sharding: head-parallel. Core i owns heads {2i, 2i+1} == E-rows [128i, 128i+128)
of Wq/Wk/Wv, and all 4 batches. No cross-core communication.

Per-core dataflow (all "transposed" layouts, d on partitions):
  qT/kT/vT [128, B*T] = W_slice @ query.T   (PE, float32r, K=E in 8 chunks)
  per (b, head, t-chunk of 512):
    scoresT [s=128, t<=512] = kT_tile.T-free matmul; causal tiles above the
    diagonal are skipped entirely, diagonal tiles are column-sliced.
    probs = Exp(scoresT + causal_mask + padding_bias)  (ACT, padding as
    per-partition bias; both heads share one [128, 1024] ACT op)
    outT[65, 512] += va_tile[128, 65].T @ probs  where va has a ones column,
    so row 64 accumulates the softmax denominator.
  normalize: DMA-broadcast denominator row, DVE reciprocal + multiply.
Host gathers [128, B, T] per-core outputs -> [T, B, E].
"""

import numpy as np

T, B, E, H = 1024, 4, 1024, 16
D = 64  # head dim
NCORES = 8
HPC = H // NCORES  # heads per core = 2
DS = HPC * D  # per-core E-slice = 128
TB = T * B
NEG = -1.0e30
SCALE = D**-0.5

_COMPILED = {}


def _build_program():
    import concourse.bacc as bacc
    import concourse.mybir as mybir
    import concourse.tile as tile
    from concourse.masks import make_identity

    f32 = mybir.dt.float32
    f32r = mybir.dt.float32r
    AF = mybir.ActivationFunctionType
    ALU = mybir.AluOpType

    nc = bacc.Bacc("TRN2", target_bir_lowering=False, debug=False,
                   num_devices=NCORES)

    qt = nc.dram_tensor("qt", [E, B, T], f32r, kind="ExternalInput").ap()
    wq = nc.dram_tensor("wq", [E, DS], f32r, kind="ExternalInput").ap()
    wk = nc.dram_tensor("wk", [E, DS], f32r, kind="ExternalInput").ap()
    wv = nc.dram_tensor("wv", [E, DS], f32r, kind="ExternalInput").ap()
    bq = nc.dram_tensor("bq", [DS, 1], f32, kind="ExternalInput").ap()
    bk = nc.dram_tensor("bk", [DS, 1], f32, kind="ExternalInput").ap()
    bv = nc.dram_tensor("bv", [DS, 1], f32, kind="ExternalInput").ap()
    cm = nc.dram_tensor("cm", [4, 128, 512], f32, kind="ExternalInput").ap()
    pad = nc.dram_tensor("pad", [128, B * 8], f32, kind="ExternalInput").ap()
    onesd = nc.dram_tensor("ones", [128, 16], f32r, kind="ExternalInput").ap()
    out = nc.dram_tensor("out", [B, T, DS], f32, kind="ExternalOutput").ap()

    NJ = TB // 512  # 8 token chunks of 512; chunk j covers (b=j//2, half=j%2)

    with tile.TileContext(nc) as tc:
        with (
            tc.tile_pool(name="consts", bufs=1) as consts,
            tc.tile_pool(name="qkv", bufs=NJ) as qkv,
            tc.tile_pool(name="va", bufs=B) as vap,
            tc.tile_pool(name="probs", bufs=3) as probsp,
            tc.tile_pool(name="outsb", bufs=3) as outsb,
            tc.tile_pool(name="bcast", bufs=3) as bcastp,
        ):
            ident = consts.tile([128, 128], f32, name="ident")
            make_identity(nc, ident[:])
            w_sb = {}
            b_sb = {}
            for nm, wdr, bdr in (("q", wq, bq), ("k", wk, bk), ("v", wv, bv)):
                wt = consts.tile([128, 8, DS], f32r, name=f"w{nm}s")
                nc.sync.dma_start(wt[:], wdr.rearrange("(c p) m -> p c m", p=128))
                w_sb[nm] = wt
                bt = consts.tile([DS, 1], f32, name=f"b{nm}s")
                nc.sync.dma_start(bt[:], bdr)
                b_sb[nm] = bt
            cm_sb = consts.tile([128, 4, 512], f32, name="cms")
            nc.sync.dma_start(cm_sb[:], cm.rearrange("d p f -> p d f"))
            pad_sb = consts.tile([128, B * 8], f32, name="pads")
            nc.sync.dma_start(pad_sb[:], pad)

            # ---- Phase 1: QKV projection into transposed layout ----
            qkv_t = {"q": [], "k": [], "v": []}
            ph1 = tc.tile_pool(name="rhs", bufs=10)
            rhsp = ph1.__enter__()
            ph1b = tc.tile_pool(name="psA", bufs=2, space="PSUM")
            psA = ph1b.__enter__()
            for j in range(NJ):
                qu = []
                for e in range(8):
                    qtile = rhsp.tile([128, 512], f32r, tag="qu", name=f"qu{j}_{e}")
                    nc.sync.dma_start(
                        qtile[:],
                        qt.rearrange("e b t -> e (b t)")[
                            e * 128:(e + 1) * 128, j * 512:(j + 1) * 512
                        ],
                    )
                    qu.append(qtile)
                for nm in ("q", "k", "v"):
                    ps = psA.tile([128, 512], f32, tag="proj", name=f"ps{nm}{j}")
                    for e in range(8):
                        nc.tensor.matmul(
                            ps[:],
                            w_sb[nm][:, e, :],
                            qu[e][:],
                            start=(e == 0),
                            stop=(e == 7),
                        )
                    dst = qkv.tile([128, 512], f32r if nm != "v" else f32, tag=f"{nm}t", name=f"{nm}t{j}")
                    nc.vector.tensor_scalar(
                        dst[:], ps[:], b_sb[nm][:], None, op0=ALU.add,
                    )
                    qkv_t[nm].append(dst)

            ph1b.__exit__(None, None, None)
            ph1.__exit__(None, None, None)

            # ---- Phase 1.5: transpose vT -> va ([s, d] with ones column) ----
            va_t = []
            for b in range(B):
                va = vap.tile([128, 8, 2, 65], f32r, tag="va", name=f"va{b}")
                nc.sync.dma_start(
                    va[:, :, :, 64:65],
                    onesd.rearrange("p (a b c) -> p a b c", a=8, b=2, c=1),
                )
                va_t.append(va)
            ph15 = tc.tile_pool(name="psTr", bufs=2, space="PSUM")
            psTr = ph15.__enter__()
            for b in range(B):
                for p in range(8):
                    j = 2 * b + p // 4
                    off = (p % 4) * 128
                    tp = psTr.tile([128, 128], f32, tag="tr", name=f"tr{b}_{p}")
                    nc.tensor.transpose(
                        tp[:], qkv_t["v"][j][:, off:off + 128], ident[:]
                    )
                    nc.vector.tensor_copy(
                        va_t[b][:, p, :, 0:64],
                        tp[:].rearrange("p (two sub) -> p two sub", two=2),
                    )
            ph15.__exit__(None, None, None)

            # ---- Phase 2: attention ----
            ph2 = tc.tile_pool(name="psS", bufs=2, space="PSUM")
            psS = ph2.__enter__()
            ph2b = tc.tile_pool(name="psO", bufs=2, space="PSUM")
            psO = ph2b.__enter__()
            ph2c = tc.tile_pool(name="psE", bufs=2, space="PSUM")
            psE = ph2c.__enter__()
            for b in range(B):
                for c in range(2):  # t-chunks of 512
                    j = 2 * b + c
                    po = []
                    for hl in range(HPC):
                        pot = psO.tile([65, 512], f32, tag="po",
                                       name=f"po{b}_{c}_{hl}")
                        po.append(pot)
                    ntile = 4 * (c + 1)
                    for p in range(ntile):
                        dlt = p - 4 * c
                        w0 = 128 * dlt if dlt > 0 else 0
                        jk = 2 * b + p // 4
                        offk = (p % 4) * 128
                        ss = psS.tile([128, 2, 512], f32, tag="sc",
                                      name=f"sc{b}_{c}_{p}")
                        for hl in range(HPC):
                            nc.tensor.matmul(
                                ss[:, hl, w0:512],
                                qkv_t["k"][jk][hl * 64:(hl + 1) * 64,
                                               offk:offk + 128],
                                qkv_t["q"][j][hl * 64:(hl + 1) * 64,
                                              w0:512],
                                start=True,
                                stop=True,
                            )
                        if dlt >= 0:
                            nc.vector.tensor_tensor(
                                ss[:, :, w0:512],
                                ss[:, :, w0:512],
                                cm_sb[:, dlt, None, w0:512].to_broadcast(
                                    (128, 2, 512 - w0)
                                ),
                                ALU.add,
                            )
                        pr = probsp.tile([128, 2, 512], f32r, tag="pr",
                                         name=f"pr{b}_{c}_{p}")
                        nc.scalar.activation(
                            pr[:, :, w0:512],
                            ss[:, :, w0:512],
                            AF.Exp,
                            bias=pad_sb[:, b * 8 + p:b * 8 + p + 1],
                            scale=1.0,
                        )
                        for hl in range(HPC):
                            nc.tensor.matmul(
                                po[hl][:, w0:512],
                                va_t[b][:, p, hl, :],
                                pr[:, hl, w0:512],
                                start=(p == 0),
                                stop=(p == ntile - 1),
                            )
                    for hl in range(HPC):
                        # epilogue: copy po to SBUF (frees the accumulator),
                        # PE-transpose to [t, d] layout, per-partition
                        # reciprocal of the denominator column, scale, store.
                        pos = bcastp.tile([65, 512], f32, tag="pos",
                                          name=f"pos{b}_{c}_{hl}")
                        nc.scalar.activation(pos[:], po[hl][:], AF.Copy)
                        te = psE.tile([128, 4, 65], f32, tag="te",
                                      name=f"te{b}_{c}_{hl}")
                        for g in range(4):
                            nc.tensor.transpose(
                                te[:, g, :],
                                pos[:, g * 128:(g + 1) * 128],
                                ident[0:65, 0:65],
                            )
                        rcp = outsb.tile([128, 4, 1], f32, tag="rcp",
                                         name=f"rcp{b}_{c}_{hl}")
                        nc.vector.reciprocal(rcp[:], te[:, :, 64:65])
                        of = outsb.tile([128, 4, 64], f32, tag="of",
                                        name=f"of{b}_{c}_{hl}")
                        for g in range(4):
                            nc.vector.tensor_scalar(
                                of[:, g, :], te[:, g, 0:64], rcp[:, g, :],
                                None, op0=ALU.mult,
                            )
                        nc.sync.dma_start(
                            out[b, c * 512:(c + 1) * 512,
                                hl * 64:(hl + 1) * 64].rearrange(
                                    "(g tp) m -> tp g m", tp=128),
                            of[:],
                        )
            ph2c.__exit__(None, None, None)
            ph2b.__exit__(None, None, None)
            ph2.__exit__(None, None, None)

    nc.compile()
    return nc


def _get_program():
    if "nc" not in _COMPILED:
        _COMPILED["nc"] = _build_program()
    return _COMPILED["nc"]


def _prepare_in_maps(query, key_padding_mask, attn_mask, Wq, bq, Wk, bk, Wv,
                     bv):
    query = np.asarray(query, dtype=np.float32)
    attn_mask = np.asarray(attn_mask, dtype=np.float32)
    kpm = np.asarray(key_padding_mask)
    Wq, Wk, Wv = (np.asarray(w, dtype=np.float32) for w in (Wq, Wk, Wv))
    bq, bk, bv = (np.asarray(x, dtype=np.float32) for x in (bq, bk, bv))

    Wq = Wq * SCALE
    bq = bq * SCALE
    qt = np.ascontiguousarray(query.transpose(2, 1, 0))  # [E, B, T]
    # causal masks for the 4 diagonal-crossing tile offsets, from attn_mask:
    # cmh[d][p, f] = attn_mask[f, 128 d + p], f in [0, 512)
    cmh = np.stack(
        [
            np.ascontiguousarray(attn_mask[:512, 128 * d:128 * (d + 1)].T)
            for d in range(4)
        ]
    )
    cmh = np.maximum(cmh, NEG)  # -inf -> -1e30 (exp underflows to exactly 0)
    padf = np.where(kpm, NEG, 0.0).astype(np.float32)  # [B, T]
    padh = np.ascontiguousarray(
        padf.reshape(B, 8, 128).transpose(2, 0, 1).reshape(128, B * 8)
    )

    in_maps = []
    for i in range(NCORES):
        rows = slice(i * DS, (i + 1) * DS)
        in_maps.append(
            {
                "qt": qt,
                "wq": np.ascontiguousarray(Wq[rows].T),
                "wk": np.ascontiguousarray(Wk[rows].T),
                "wv": np.ascontiguousarray(Wv[rows].T),
                "bq": np.ascontiguousarray(bq[rows, None]),
                "bk": np.ascontiguousarray(bk[rows, None]),
                "bv": np.ascontiguousarray(bv[rows, None]),
                "cm": cmh,
                "pad": padh,
                "ones": np.ones((128, 16), dtype=np.float32),
            }
        )
    return in_maps


def kernel(query, key, key_padding_mask, attn_mask, Wq, bq, Wk, bk, Wv, bv,
           num_heads):
    from concourse.bass_utils import run_bass_kernel_spmd

    assert int(num_heads) == H
    nc = _get_program()
    in_maps = _prepare_in_maps(query, key_padding_mask, attn_mask, Wq, bq, Wk,
                               bk, Wv, bv)
    res = run_bass_kernel_spmd(nc, in_maps, core_ids=list(range(NCORES)))
    full = np.concatenate(
        [res.results[i]["out"] for i in range(NCORES)], axis=2
    ).transpose(1, 0, 2)
    return np.ascontiguousarray(full)
